# revision 5
# baseline (speedup 1.0000x reference)
"""DAGNN (gnn_message_passing) Trainium2 kernel, 8-core SPMD.

Strategy:
- Nodes sharded by receiver into 8 contiguous blocks (12500/core).
- MLP head (relu(X@W1+b1)@W2+b2) computed node-parallel on-device from a
  host-pretransposed X^T input.
- 20 SpMM hops: edges receiver-sorted and packed into 128-slot tiles that
  never split a receiver (<=8 receivers/tile). Per tile, one indirect-DMA
  gather pulls the 128 sender rows, and one PE matmul with a [128, 8]
  weight-folded indicator computes the per-receiver partial sums into
  PSUM at 32-aligned bases. Outputs are kept in a tile-compact row order
  so all stores stay affine; the next hop's gather indices point at the
  compact positions, avoiding any hardware scatter. An 8-core AllGather
  rebuilds the full (compact) node state each hop.
- The gated sum acc += sigmoid(x_k@Wg+bg)*x_k is accumulated on-chip
  (node order for hop 0, compact order for hops 1..20); the host adds the
  two accumulators and un-permutes the compact rows at the end.
"""
import numpy as np

import concourse.bacc as bacc
import concourse.bass as bass
import concourse.mybir as mybir
import concourse.tile as tile
from concourse.bass_utils import run_bass_kernel_spmd

NCORES = 8

N_NODES = 100000
N_EDGES = 3200000
F_IN = 512
HIDDEN = 64
NCLS = 64
HOPS = 20

R = 8              # max receivers per tile
RW = 32            # indicator width (32-aligned PSUM granule)
TPP = 32           # tiles per psum tile [128, 512] (4 bases x 8 col slots)


def _pack_tiles(deg, max_r=R, cap=128):
    tiles = []
    cur = []
    fill = 0
    for rid, d in enumerate(deg):
        d = int(d)
        if fill + d > cap or len(cur) >= max_r:
            tiles.append(cur)
            cur = []
            fill = 0
        cur.append(rid)
        fill += d
    if cur:
        tiles.append(cur)
    return tiles


def _xcrow(t, j):
    """Physical row of compact slot (tile t, col j) in xc_shard."""
    P, u = divmod(t, TPP)
    a, sl = u % 4, u // 4
    return (P * 128 + 32 * a + j) * 8 + sl


def _preprocess(senders, receivers, weights, n_nodes, shard, n_cores):
    order = np.argsort(receivers, kind="stable")
    ss, ws = senders[order], weights[order]
    deg = np.bincount(receivers, minlength=n_nodes)
    starts = np.zeros(n_nodes + 1, np.int64)
    np.cumsum(deg, out=starts[1:])

    cores = []
    for c in range(n_cores):
        lo = c * shard
        cores.append(_pack_tiles(deg[lo:lo + shard]))
    n_tiles = max(len(t) for t in cores)
    n_tiles = ((n_tiles + TPP - 1) // TPP) * TPP
    ncrows = n_tiles * RW
    npsum = n_tiles // TPP

    gidx = np.zeros((n_cores, 128, n_tiles), np.int32)
    m_all = np.zeros((n_cores, npsum, 128, TPP * RW), np.float32)
    cpos = np.zeros(n_nodes, np.int64)
    for c in range(n_cores):
        lo = c * shard
        for t, recvs in enumerate(cores[c]):
            p = 0
            P, u = divmod(t, TPP)
            for j, rid in enumerate(recvs):
                g = lo + rid
                cpos[g] = c * ncrows + _xcrow(t, j)
                a, b = starts[g], starts[g + 1]
                n = b - a
                gidx[c, p:p + n, t] = ss[a:b]
                m_all[c, P, p:p + n, u * RW + j] = ws[a:b]
                p += n
    gidxc = cpos[gidx].astype(np.int32)
    return {
        "n_tiles": n_tiles, "ncrows": ncrows,
        "gidx": gidx, "gidxc": gidxc, "m_all": m_all, "cores": cores,
    }


def _build(n_nodes, shard, n_tiles, ncrows, f_in, hops):
    nc = bacc.Bacc("TRN2", target_bir_lowering=False, debug=False,
                   num_devices=NCORES)
    f32 = mybir.dt.float32
    i32 = mybir.dt.int32
    npsum = n_tiles // TPP
    ntile_node = (shard + 127) // 128
    nk = f_in // 128

    xt = nc.dram_tensor("xt", [f_in, shard], f32, kind="ExternalInput")
    w1 = nc.dram_tensor("w1", [f_in, HIDDEN], f32, kind="ExternalInput")
    b1 = nc.dram_tensor("b1", [HIDDEN, 1], f32, kind="ExternalInput")
    w2 = nc.dram_tensor("w2", [HIDDEN, NCLS], f32, kind="ExternalInput")
    b2r = nc.dram_tensor("b2r", [128, NCLS], f32, kind="ExternalInput")
    wgr = nc.dram_tensor("wgr", [128, 512], f32, kind="ExternalInput")
    bgr = nc.dram_tensor("bgr", [128, 1], f32, kind="ExternalInput")
    gidx_d = nc.dram_tensor("gidx", [128, n_tiles], i32, kind="ExternalInput")
    gidxc_d = nc.dram_tensor("gidxc", [128, n_tiles], i32, kind="ExternalInput")
    m_d = nc.dram_tensor("m", [npsum, 128, TPP * RW], f32,
                         kind="ExternalInput")

    out0 = nc.dram_tensor("out0", [shard, NCLS], f32, kind="ExternalOutput")
    outc = nc.dram_tensor("outc", [npsum, 128, 512], f32,
                          kind="ExternalOutput")

    with tile.TileContext(nc) as tc:
        with (
            tc.tile_pool(name="const", bufs=1) as cp,
            tc.tile_pool(name="mlp", bufs=3) as mp,
            tc.tile_pool(name="mpsum", bufs=2, space="PSUM") as mps,
            tc.tile_pool(name="gp", bufs=16) as gp,
            tc.tile_pool(name="spsum", bufs=4, space="PSUM") as sps,
            tc.tile_pool(name="stage", bufs=3) as stp,
            tc.tile_pool(name="mm", bufs=3) as mmp,
            tc.tile_pool(name="dram", bufs=1, space="DRAM") as dr,
        ):
            w1_t = cp.tile([128, nk * HIDDEN], f32)
            for k in range(nk):
                nc.sync.dma_start(w1_t[:, k * HIDDEN:(k + 1) * HIDDEN],
                                  w1[k * 128:(k + 1) * 128, :])
            b1_t = cp.tile([HIDDEN, 1], f32)
            nc.sync.dma_start(b1_t[:], b1[:, :])
            w2_t = cp.tile([HIDDEN, NCLS], f32)
            nc.sync.dma_start(w2_t[:], w2[:, :])
            b2_t = cp.tile([128, NCLS], f32)
            nc.sync.dma_start(b2_t[:], b2r[:, :])
            wg_t = cp.tile([128, 512], f32)
            nc.sync.dma_start(wg_t[:], wgr[:, :])
            bg_t = cp.tile([128, 1], f32)
            nc.sync.dma_start(bg_t[:], bgr[:, :])
            gidx_t = cp.tile([128, n_tiles], i32)
            nc.sync.dma_start(gidx_t[:], gidx_d[:, :])
            gidxc_t = cp.tile([128, n_tiles], i32)
            nc.sync.dma_start(gidxc_t[:], gidxc_d[:, :])

            acc0 = cp.tile([128, ntile_node * NCLS], f32)
            nc.vector.memset(acc0[:], 0.0)

            x_shard = dr.tile([shard, NCLS], f32)
            x_node = dr.tile([n_nodes, NCLS], f32, addr_space="Shared")
            xc_shard = dr.tile([ncrows, NCLS], f32)
            xc_fulls = [
                dr.tile([NCORES * ncrows, NCLS], f32, addr_space="Shared",
                        name=f"xc_full_{i}")
                for i in range(hops - 1)
            ]

            # ---- MLP phase ----
            for i in range(ntile_node):
                nn = min(128, shard - i * 128)
                hps = mps.tile([HIDDEN, 128], f32, tag="hps")
                for k in range(nk):
                    xt_t = mp.tile([128, 128], f32, tag="xt")
                    nc.sync.dma_start(
                        xt_t[:, :nn],
                        xt[k * 128:(k + 1) * 128, i * 128:i * 128 + nn])
                    nc.tensor.matmul(
                        out=hps[:, :nn],
                        lhsT=w1_t[:, k * HIDDEN:(k + 1) * HIDDEN],
                        rhs=xt_t[:, :nn],
                        start=(k == 0), stop=(k == nk - 1))
                h_t = mp.tile([HIDDEN, 128], f32, tag="h")
                nc.scalar.activation(
                    h_t[:, :nn], hps[:, :nn],
                    mybir.ActivationFunctionType.Relu, bias=b1_t[:])
                xps = mps.tile([128, NCLS], f32, tag="xps")
                nc.tensor.matmul(out=xps[:nn, :], lhsT=h_t[:, :nn],
                                 rhs=w2_t[:], start=True, stop=True)
                x0 = mp.tile([128, NCLS], f32, tag="x0")
                nc.vector.tensor_add(x0[:nn, :], xps[:nn, :], b2_t[:nn, :])
                nc.sync.dma_start(x_shard[i * 128:i * 128 + nn, :], x0[:nn, :])
                tmp = mp.tile([128, NCLS], f32, tag="gt")
                nc.vector.tensor_mul(tmp[:nn, :], x0[:nn, :],
                                     wg_t[:nn, :NCLS])
                sg = mp.tile([128, 1], f32, tag="sg")
                nc.vector.reduce_sum(sg[:nn, :], tmp[:nn, :],
                                     axis=mybir.AxisListType.X)
                nc.scalar.activation(sg[:nn, :], sg[:nn, :],
                                     mybir.ActivationFunctionType.Sigmoid,
                                     bias=bg_t[:nn, :])
                nc.vector.tensor_mul(tmp[:nn, :], x0[:nn, :],
                                     sg[:nn, :].to_broadcast([nn, NCLS]))
                nc.vector.tensor_add(acc0[:nn, i * NCLS:(i + 1) * NCLS],
                                     acc0[:nn, i * NCLS:(i + 1) * NCLS],
                                     tmp[:nn, :])

            # ---- hops ----
            for k in range(1, hops + 1):
                if k == 1:
                    nc.gpsimd.collective_compute(
                        "AllGather", mybir.AluOpType.bypass,
                        replica_groups=[list(range(NCORES))],
                        ins=[x_shard[:, :]], outs=[x_node[:, :]])
                    src_ap = x_node[:, :]
                    idx_t = gidx_t
                else:
                    xc_full = xc_fulls[k - 2]
                    nc.gpsimd.collective_compute(
                        "AllGather", mybir.AluOpType.bypass,
                        replica_groups=[list(range(NCORES))],
                        ins=[xc_shard[:, :]], outs=[xc_full[:, :]])
                    src_ap = xc_full[:, :]
                    idx_t = gidxc_t

                for P in range(npsum):
                    ps = sps.tile([128, 512], f32, tag="ps")
                    m_t = mmp.tile([128, TPP * RW], f32, tag="m")
                    nc.sync.dma_start(m_t[:], m_d[P, :, :])
                    for u in range(TPP):
                        t = P * TPP + u
                        a, sl = u % 4, u // 4
                        g = gp.tile([128, NCLS], f32, tag="g")
                        nc.gpsimd.indirect_dma_start(
                            out=g[:], out_offset=None,
                            in_=src_ap,
                            in_offset=bass.IndirectOffsetOnAxis(
                                ap=idx_t[:, t:t + 1], axis=0))
                        nc.tensor.matmul(
                            out=ps[32 * a:32 * a + RW,
                                   sl * NCLS:(sl + 1) * NCLS],
                            lhsT=m_t[:, u * RW:(u + 1) * RW],
                            rhs=g[:],
                            start=True, stop=True,
                            tile_position=(0, 32 * a))
                    stg = stp.tile([128, 512], f32, tag="stg")
                    nc.vector.tensor_copy(stg[:], ps[:])
                    tmp = stp.tile([128, 512], f32, tag="gtmp")
                    nc.vector.tensor_mul(tmp[:], stg[:], wg_t[:])
                    s8 = stp.tile([128, 8], f32, tag="s8")
                    nc.vector.reduce_sum(
                        s8[:], tmp[:].rearrange("p (e c) -> p e c", c=NCLS),
                        axis=mybir.AxisListType.X)
                    nc.scalar.activation(s8[:], s8[:],
                                         mybir.ActivationFunctionType.Sigmoid,
                                         bias=bg_t[:])
                    sb = s8[:].rearrange("p (e one) -> p e one", one=1)
                    nc.vector.tensor_mul(
                        tmp[:].rearrange("p (e c) -> p e c", c=NCLS),
                        stg[:].rearrange("p (e c) -> p e c", c=NCLS),
                        sb.to_broadcast([128, 8, NCLS]))
                    if k == 1:
                        nc.sync.dma_start(outc[P, :, :], tmp[:])
                    else:
                        acs = stp.tile([128, 512], f32, tag="acs")
                        nc.sync.dma_start(acs[:], outc[P, :, :])
                        nc.vector.tensor_add(acs[:], acs[:], tmp[:])
                        nc.sync.dma_start(outc[P, :, :], acs[:])
                    if k < hops:
                        nc.sync.dma_start(
                            xc_shard[:, :].rearrange(
                                "(P p e) c -> P p (e c)",
                                P=npsum, p=128)[P, :, :],
                            stg[:])

            for i in range(ntile_node):
                nn = min(128, shard - i * 128)
                nc.sync.dma_start(out0[i * 128:i * 128 + nn, :],
                                  acc0[:nn, i * NCLS:(i + 1) * NCLS])

    nc.finalize()
    return nc


def kernel(node_features, edge_weights, W1, b1, W2, b2, Wg, bg,
           senders, receivers):
    node_features = np.asarray(node_features, np.float32)
    edge_weights = np.asarray(edge_weights, np.float32)
    W1 = np.asarray(W1, np.float32)
    b1v = np.asarray(b1, np.float32)
    W2 = np.asarray(W2, np.float32)
    b2v = np.asarray(b2, np.float32)
    Wgv = np.asarray(Wg, np.float32).reshape(-1)
    bgv = np.asarray(bg, np.float32).reshape(-1)
    senders = np.asarray(senders, np.int32)
    receivers = np.asarray(receivers, np.int32)

    n_nodes, f_in = node_features.shape
    shard = n_nodes // NCORES
    pre = _preprocess(senders, receivers, edge_weights, n_nodes, shard,
                      NCORES)
    n_tiles, ncrows = pre["n_tiles"], pre["ncrows"]

    nc = _build(n_nodes, shard, n_tiles, ncrows, f_in, HOPS)

    wg_rep = np.tile(Wgv[None, :], (128, 8)).astype(np.float32)
    bg_rep = np.full((128, 1), bgv[0], np.float32)
    b2_rep = np.tile(b2v[None, :], (128, 1)).astype(np.float32)
    b1_col = b1v[:, None].astype(np.float32)

    in_maps = []
    for c in range(NCORES):
        lo = c * shard
        in_maps.append({
            "xt": np.ascontiguousarray(node_features[lo:lo + shard].T),
            "w1": W1, "b1": b1_col, "w2": W2, "b2r": b2_rep,
            "wgr": wg_rep, "bgr": bg_rep,
            "gidx": pre["gidx"][c], "gidxc": pre["gidxc"][c],
            "m": pre["m_all"][c],
        })

    res = run_bass_kernel_spmd(nc, in_maps, core_ids=list(range(NCORES)))

    out = np.zeros((n_nodes, NCLS), np.float32)
    for c in range(NCORES):
        lo = c * shard
        r = res.results[c]
        out[lo:lo + shard] = r["out0"]
        oc = r["outc"]  # [npsum, 128, 512]
        for t, recvs in enumerate(pre["cores"][c]):
            P, u = divmod(t, TPP)
            a, sl = u % 4, u // 4
            for j, rid in enumerate(recvs):
                out[lo + rid] += oc[P, 32 * a + j,
                                    sl * NCLS:(sl + 1) * NCLS]
    return out
